# revision 15
# baseline (speedup 1.0000x reference)
"""Self-contained Trainium2 Bass kernel: GRU(relu, reset_after) + BN + Dense.

kernel(**inputs) takes FULL unsharded fp32 inputs, shards batch over 8
NeuronCores, runs the Bass kernel via run_bass_kernel_spmd, returns the
FULL [2048, 1] fp32 output.

Layout (per core):
  B=256 batch (2 chunks of Bc=128), T timesteps, F=32 in-features, H=256 hidden.
  Transposed: hidden on partitions, batch on free dim.
  h[c][p, ct*128+n] = h_state[batch c*128+n, hidden ct*128+p]

DRAM inputs (host-prepped):
  xT  [T/2*128, 256] f16  tile j: rows 0:32 = x[2j] feats, row 32 = 1.0,
                          rows 64:96 = x[2j+1] feats, row 96 = 1.0, rest 0.
  wi  [128, 1024] f16     rows 0:32 cols 0:768 = kernel (gate block m at
                          cols m*128); row 32 cols 0:768 = [b_z|b_r|b_xh];
                          row 32 cols 768:1024 = [b_rh0|b_rh1]; rows 64:97
                          replicate rows 0:33 (for odd-timestep row group).
  wr8 [128, 1536] f8e4    rec weights: [:, m*256+k*128+j] = rec[k*128+p, m*128+j]
                          (DoubleRow k-tile layout, K=256 in one matmul)
  sv  [128, 2] f16        BN+dense folded scale; cv [1,1] f32 folded bias.
Output:
  y   [1, 256] f32        per-core output slice.

All gate biases are baked into the PSUM accumulation by the x-projection
matmuls (K=33: 32 features + a ones-row whose stationary row carries the
bias), and b_rh via two K=1 matmuls — no bias work on vector engines.

Gate math per step (z/r/xh/rh pre-acts accumulated in PSUM):
  r = sigmoid(psum_r); z = sigmoid(psum_z); w = sigmoid(-psum_z) = 1-z
  p = psum_rh * r ; s = psum_xh + p
  u = z*h ; v = relu(s)*w ; h_new = v + u ; h8 = fp8(h_new)
"""
from contextlib import ExitStack

import numpy as np

import concourse.bass as bass
import concourse.tile as tile
from concourse import bacc, mybir

F16 = mybir.dt.float16
F32 = mybir.dt.float32
F8 = mybir.dt.float8e4
AF = mybir.ActivationFunctionType
OP = mybir.AluOpType
DR = mybir.MatmulPerfMode.DoubleRow


def build_gru_nc(T=256, debug=False):
    nc = bacc.Bacc("TRN2", num_devices=8, debug=debug)
    xT_d = nc.dram_tensor("xT", [T // 2 * 128, 256], F16, kind="ExternalInput")
    wi_d = nc.dram_tensor("wi", [128, 1024], F16, kind="ExternalInput")
    wr8_d = nc.dram_tensor("wr8", [128, 1536], F8, kind="ExternalInput")
    sv_d = nc.dram_tensor("sv", [128, 2], F16, kind="ExternalInput")
    cv_d = nc.dram_tensor("cv", [1, 1], F32, kind="ExternalInput")
    eye_d = nc.dram_tensor("eye", [128, 128], F16, kind="ExternalInput")
    y_d = nc.dram_tensor("y", [1, 256], F32, kind="ExternalOutput")

    with tile.TileContext(nc) as tc, ExitStack() as ctx:
        const = ctx.enter_context(tc.tile_pool(name="const", bufs=1))
        hpool = [
            ctx.enter_context(tc.tile_pool(name=f"h{c}", bufs=2)) for c in (0, 1)
        ]
        h8pool = [
            ctx.enter_context(tc.tile_pool(name=f"h8{c}", bufs=2)) for c in (0, 1)
        ]
        gpool = [
            ctx.enter_context(tc.tile_pool(name=f"g{c}", bufs=2)) for c in (0, 1)
        ]
        # paired PSUM tiles spanning both chunks: cols c*512 + (blk*128 + n)
        zrpool = ctx.enter_context(
            tc.tile_pool(name="zrp", bufs=2, space=bass.MemorySpace.PSUM)
        )
        xrpool = ctx.enter_context(
            tc.tile_pool(name="xrp", bufs=2, space=bass.MemorySpace.PSUM)
        )

        ntile = T // 2  # [128, 256] x-tiles, one per 2 timesteps
        xsb = const.tile([128, ntile * 256], F16)
        wi = const.tile([128, 1024], F16)
        wr8 = const.tile([128, 1536], F8)
        sv = const.tile([128, 2], F16)
        cv = const.tile([1, 1], F32)

        nc.sync.dma_start(wi[:], wi_d.ap())
        nc.sync.dma_start(wr8[:], wr8_d.ap())
        nc.sync.dma_start(sv[:], sv_d.ap())
        nc.sync.dma_start(cv[:], cv_d.ap())

        nchunk = 4
        per = ntile // nchunk
        for jc in range(nchunk):
            src = xT_d.ap()[jc * per * 128 : (jc + 1) * per * 128, :]
            src = src.rearrange("(j p) b -> p j b", p=128)
            dst = xsb[:, jc * per * 256 : (jc + 1) * per * 256]
            dst = dst.rearrange("p (j b) -> p j b", b=256)
            nc.sync.dma_start(dst, src)

        h, h8 = [], []
        for c in (0, 1):
            h0 = hpool[c].tile([128, 256], F16)
            nc.vector.memset(h0[:], 0.0)
            h.append(h0)
            h80 = h8pool[c].tile([128, 256], F8)
            nc.gpsimd.memset(h80[:], 0.0)
            h8.append(h80)

        def x_phase(t):
            """x-projection + all-bias matmuls for step t, BOTH chunks at
            once (N=256 strided out across the chunk-paired psum tiles)."""
            zr = zrpool.tile([128, 1024], F32)
            xr = xrpool.tile([128, 1024], F32)
            zrs = zr[:].rearrange("p (c q) -> p c q", c=2)
            xrs = xr[:].rearrange("p (c q) -> p c q", c=2)
            base = 64 * (t % 2)
            col0 = (t // 2) * 256
            xrhs = xsb[base : base + 33, col0 : col0 + 256]
            for m in range(6):
                lhsT = wi[base : base + 33, m * 128 : (m + 1) * 128]
                if m < 4:
                    out = zrs[:, :, m * 128 : (m + 1) * 128]
                else:
                    out = xrs[:, :, (m - 4) * 128 : (m - 3) * 128]
                if m in (0, 4):
                    # bank-clearing first write: start=True clears only the
                    # bank containing the out region, so issue per-chunk
                    tgt = zr if m == 0 else xr
                    for cc in (0, 1):
                        nc.tensor.matmul(
                            tgt[:, cc * 512 : cc * 512 + 128],
                            lhsT,
                            xsb[base : base + 33, col0 + cc * 128 : col0 + (cc + 1) * 128],
                            start=True, stop=False, tile_position=(base, 0),
                        )
                else:
                    nc.tensor.matmul(
                        out, lhsT, xrhs, start=False, stop=False,
                        tile_position=(base, 0),
                    )
            # b_rh into the rh regions via K=1 mm against the ones-row
            ones = xsb[base + 32 : base + 33, col0 : col0 + 256]
            for ct in (0, 1):
                lhsT = wi[base + 32 : base + 33, 768 + ct * 128 : 768 + (ct + 1) * 128]
                nc.tensor.matmul(
                    xrs[:, :, 256 + ct * 128 : 256 + (ct + 1) * 128], lhsT, ones,
                    start=False, stop=False, tile_position=(base + 32, 0),
                )
            return zr, xr

        eye = const.tile([128, 128], F16)
        nc.sync.dma_start(eye[:], eye_d.ap())
        cur = x_phase(0)
        nxt = x_phase(1)
        pending_cast = None  # (hn_tile, chunk) awaiting fp8 cast on scalar

        for t in range(T):
            zr, xr = cur
            for c in (0, 1):
                o = c * 512
                hc, h8c = h[c], h8[c]
                rhs8 = h8c[:].rearrange("p (k n) -> p k n", k=2)
                # rec matmuls, fp8 DoubleRow (K=256 in one pass): r blocks
                # first so sigmoid(r) fires early, then rh, z last.
                for m in (2, 3, 4, 5, 0, 1):
                    if m < 4:
                        out = zr[:, o + m * 128 : o + (m + 1) * 128]
                    else:
                        out = xr[:, o + 256 + (m - 4) * 128 : o + 256 + (m - 3) * 128]
                    lhsT = wr8[:, m * 256 : (m + 1) * 256].rearrange(
                        "p (k j) -> p k j", k=2
                    )
                    nc.tensor.matmul(
                        out, lhsT, rhs8,
                        start=False, stop=(m in (1, 5)), perf_mode=DR,
                    )
                # advance the x lookahead here so its PE work sits between
                # rec(c1) and the late id-mm in the queue
                if c == 1:
                    cur = nxt
                    if t + 2 < T:
                        nxt = x_phase(t + 2)

                # deferred fp8 cast of the OTHER chunk's h first: it overlaps
                # this chunk's rec matmuls and unblocks the other chunk's rec
                if pending_cast is not None:
                    hn_prev, cprev = pending_cast
                    h8n = h8pool[cprev].tile([128, 256], F8)
                    nc.scalar.copy(h8n[:], hn_prev[:])
                    h8[cprev] = h8n
                    pending_cast = None
                # sigmoid r first (path-critical), z after; 1-z via tensor op
                r_sb = gpool[c].tile([128, 256], F16, tag="r")
                nc.scalar.activation(r_sb[:], zr[:, o + 256 : o + 512], AF.Sigmoid)
                z_sb = gpool[c].tile([128, 256], F16, tag="z")
                nc.scalar.activation(z_sb[:], zr[:, o : o + 256], AF.Sigmoid)

                w_sb = gpool[c].tile([128, 256], F16, tag="w")
                nc.gpsimd.tensor_scalar(w_sb[:], z_sb[:], -1.0, 1.0, OP.mult, OP.add)
                u = gpool[c].tile([128, 256], F16, tag="u")
                nc.gpsimd.tensor_tensor(u[:], z_sb[:], hc[:], OP.mult)

                p = gpool[c].tile([128, 256], F16, tag="p")
                nc.vector.tensor_tensor(p[:], xr[:, o + 256 : o + 512], r_sb[:], OP.mult)
                # s = xh + p computed BY THE PE: identity-matmul accumulates p
                # onto the xh psum region (has_written persists across groups)
                nc.tensor.matmul(
                    xr[:, o : o + 256], eye[:], p[:],
                    start=False, stop=True, skip_group_check=True,
                )
                # q = s * w ; hn = relu(q) + u  (= relu(s)*(1-z) + z*h)
                q = gpool[c].tile([128, 256], F16, tag="q")
                nc.vector.tensor_tensor(q[:], xr[:, o : o + 256], w_sb[:], OP.mult)
                hn = hpool[c].tile([128, 256], F16)
                nc.vector.scalar_tensor_tensor(
                    hn[:], q[:], 0.0, u[:], op0=OP.max, op1=OP.add
                )
                h[c] = hn
                pending_cast = (hn, c)

        # flush the last pending cast (h8 unused afterwards, but keep state
        # consistent)
        if pending_cast is not None:
            hn_prev, cprev = pending_cast
            h8n = h8pool[cprev].tile([128, 256], F8)
            nc.scalar.copy(h8n[:], hn_prev[:])
            h8[cprev] = h8n
            pending_cast = None

        # ---- BN + dense epilogue: y = s . h + c (reuse last x tile) ----
        fin = cur[0]
        first = True
        for c in (0, 1):
            for ct in (0, 1):
                nc.tensor.matmul(
                    fin[0:1, c * 128 : (c + 1) * 128],
                    sv[:, ct : ct + 1],
                    h[c][:, ct * 128 : (ct + 1) * 128],
                    start=first,
                    stop=(c == 1 and ct == 1),
                )
                first = False
        ysb = const.tile([1, 256], F32)
        nc.vector.tensor_scalar_add(ysb[:], fin[0:1, 0:256], cv[0:1, 0:1])
        nc.sync.dma_start(y_d.ap(), ysb[:])

    nc.compile()
    return nc


BN_EPS = 1e-3


def prep_core_inputs(x_core, kernel, rec_kernel, bias, gamma, beta,
                     moving_mean, moving_var, dense_w, dense_b):
    """Host-side prep of one core's input dict. x_core: [B=256, T, 32] f32."""
    import ml_dtypes

    B, T, F = x_core.shape
    H = 256
    # xT tiles: 2 timesteps per 128 rows, feats at rows 0:32 / 64:96,
    # ones-row at 32 / 96.
    xt = np.zeros((T // 2, 128, B), np.float16)
    xf = x_core.astype(np.float16).transpose(1, 2, 0)  # [T, 32, B]
    xt[:, 0:32, :] = xf[0::2]
    xt[:, 64:96, :] = xf[1::2]
    xt[:, 32, :] = 1.0
    xt[:, 96, :] = 1.0
    xT = np.ascontiguousarray(xt.reshape(T // 2 * 128, B))

    b_z = bias[0, 0:256] + bias[1, 0:256]
    b_r = bias[0, 256:512] + bias[1, 256:512]
    b_xh = bias[0, 512:768]
    b_rh = bias[1, 512:768]
    wi = np.zeros((128, 1024), np.float16)
    wi[0:32, 0:768] = kernel.astype(np.float16)
    wi[32, 0:768] = np.concatenate([b_z, b_r, b_xh]).astype(np.float16)
    wi[32, 768:1024] = b_rh.astype(np.float16)
    wi[64:97, :] = wi[0:33, :]

    rec8 = np.clip(rec_kernel, -240, 240).astype(ml_dtypes.float8_e4m3)
    wr8 = np.zeros((128, 1536), ml_dtypes.float8_e4m3)
    for m in range(6):
        for k in (0, 1):
            wr8[:, m * 256 + k * 128 : m * 256 + (k + 1) * 128] = rec8[
                k * 128 : (k + 1) * 128, m * 128 : (m + 1) * 128
            ]

    rs = 1.0 / np.sqrt(moving_var + BN_EPS)
    s = (gamma * rs * dense_w[:, 0]).astype(np.float16)
    sv = np.stack([s[:128], s[128:]], axis=1)
    cc = dense_b[0] + np.sum((beta - moving_mean * gamma * rs) * dense_w[:, 0])
    cv = np.array([[cc]], np.float32)
    return {
        "xT": xT,
        "wi": np.ascontiguousarray(wi),
        "wr8": np.ascontiguousarray(wr8),
        "sv": np.ascontiguousarray(sv),
        "cv": cv,
        "eye": np.eye(128, dtype=np.float16),
    }


_NC_CACHE = {}


def _get_nc():
    if "nc" not in _NC_CACHE:
        _NC_CACHE["nc"] = build_gru_nc(T=256)
    return _NC_CACHE["nc"]


def kernel(x, kernel, rec_kernel, bias, gamma, beta, moving_mean, moving_var,
           dense_w, dense_b):
    from concourse.bass_utils import run_bass_kernel_spmd

    x = np.asarray(x, dtype=np.float32)
    args = [np.asarray(a, dtype=np.float32) for a in
            (kernel, rec_kernel, bias, gamma, beta, moving_mean, moving_var,
             dense_w, dense_b)]
    nc = _get_nc()
    n_cores = 8
    nb = x.shape[0] // n_cores
    in_maps = [prep_core_inputs(x[i * nb : (i + 1) * nb], *args)
               for i in range(n_cores)]
    res = run_bass_kernel_spmd(nc, in_maps, core_ids=list(range(n_cores)))
    return np.concatenate(
        [res.results[i]["y"].reshape(nb, 1) for i in range(n_cores)], axis=0
    ).astype(np.float32)


# revision 16
# speedup vs baseline: 1.1264x; 1.1264x over previous
"""Self-contained Trainium2 Bass kernel: GRU(relu, reset_after) + BN + Dense.

kernel(**inputs) takes FULL unsharded fp32 inputs, shards batch over 8
NeuronCores, runs the Bass kernel via run_bass_kernel_spmd, returns the
FULL [2048, 1] fp32 output.
"""
import numpy as np

"""GRU Bass kernel builder for TRN2 (one NeuronCore program, SPMD over 8 cores).

Layout (per core):
  B=256 batch (2 chunks of Bc=128), T timesteps, F=32 in-features, H=256 hidden.
  Everything transposed: H on partitions, batch on free dim.

DRAM inputs (host-prepped, fp16 unless noted):
  xT   [T*32, 256]  x transposed+interleaved: row t*32+f, col b
  wr   [128, 1536]  rec_kernel lhsT tiles: col block (c*6+m)*128 = rec[c*128:., m*128:.]
  wi   [128, 768]   input kernel replicated at partition groups 0/32/64/96
  bz   [128, 1024]  bias rows at partitions {0,32,64,96}; col block j = b_all[j*128:(j+1)*128]
                    b_all = [b_z(256) | b_r(256) | b_xh(256) | b_rh(256)]
  sv   [128, 2]     BN+dense folded scale s: col c = s[c*128:(c+1)*128]
  cv   [1, 1] f32   scalar constant folded from BN/dense biases
Output:
  y    [1, 256] f32  per-core output slice (before host concat)
"""
from contextlib import ExitStack

import concourse.bass as bass
import concourse.tile as tile
from concourse import bacc, mybir

F16 = mybir.dt.float16
F32 = mybir.dt.float32
AF = mybir.ActivationFunctionType


def build_gru_nc(T=256, debug=False):
    nc = bacc.Bacc("TRN2", num_devices=8, debug=debug)
    xT_d = nc.dram_tensor("xT", [T * 32, 256], F16, kind="ExternalInput")
    wr_d = nc.dram_tensor("wr", [128, 1536], F16, kind="ExternalInput")
    wi_d = nc.dram_tensor("wi", [128, 768], F16, kind="ExternalInput")
    bz_d = nc.dram_tensor("bz", [128, 1024], F16, kind="ExternalInput")
    sv_d = nc.dram_tensor("sv", [128, 2], F16, kind="ExternalInput")
    cv_d = nc.dram_tensor("cv", [1, 1], F32, kind="ExternalInput")
    brh_d = nc.dram_tensor("brh", [128, 2], F32, kind="ExternalInput")
    bxh_d = nc.dram_tensor("bxh", [128, 2], F32, kind="ExternalInput")
    y_d = nc.dram_tensor("y", [1, 256], F32, kind="ExternalOutput")

    with tile.TileContext(nc) as tc, ExitStack() as ctx:
        const = ctx.enter_context(tc.tile_pool(name="const", bufs=1))
        hpool = [
            ctx.enter_context(tc.tile_pool(name=f"h{c}", bufs=2)) for c in (0, 1)
        ]
        gpool = [
            ctx.enter_context(tc.tile_pool(name=f"g{c}", bufs=2)) for c in (0, 1)
        ]
        zrpool = [
            ctx.enter_context(
                tc.tile_pool(name=f"zr{c}", bufs=2, space=bass.MemorySpace.PSUM)
            )
            for c in (0, 1)
        ]
        xrpool = [
            ctx.enter_context(
                tc.tile_pool(name=f"xr{c}", bufs=2, space=bass.MemorySpace.PSUM)
            )
            for c in (0, 1)
        ]

        # ---- constants / weights ----
        xsb = const.tile([128, (T * 32 // 128) * 256], F16)  # x^T tiles, col blk j
        wr = const.tile([128, 1536], F16)
        wi = const.tile([128, 768], F16)
        bz = const.tile([128, 1024], F16)
        sv = const.tile([128, 2], F16)
        cv = const.tile([1, 1], F32)
        brh = const.tile([128, 2], F32)
        bxh = const.tile([128, 2], F32)
        ones = const.tile([128, 128], F16)

        nc.sync.dma_start(wr[:], wr_d.ap())
        nc.sync.dma_start(wi[:], wi_d.ap())
        nc.sync.dma_start(bz[:], bz_d.ap())
        nc.sync.dma_start(sv[:], sv_d.ap())
        nc.sync.dma_start(cv[:], cv_d.ap())
        nc.sync.dma_start(brh[:], brh_d.ap())
        nc.sync.dma_start(bxh[:], bxh_d.ap())
        nc.vector.memset(ones[:], 1.0)

        # x load: split into up to 4 chunks of row-tiles for pipelining
        ntile = T * 32 // 128  # 64 for T=256
        nchunk = min(4, ntile)
        per = ntile // nchunk
        for jc in range(nchunk):
            src = xT_d.ap()[jc * per * 128 : (jc + 1) * per * 128, :]
            src = src.rearrange("(j p) b -> p j b", p=128)
            dst = xsb[:, jc * per * 256 : (jc + 1) * per * 256]
            dst = dst.rearrange("p (j b) -> p j b", b=256)
            nc.sync.dma_start(dst, src)

        # ---- initial hidden state ----
        h = []
        for c in (0, 1):
            h0 = hpool[c].tile([128, 256], F16)
            nc.vector.memset(h0[:], 0.0)
            h.append(h0)

        def x_phase(t, c):
            """Emit x-projection + bias matmuls for step t, chunk c."""
            zr = zrpool[c].tile([128, 512], F32)
            xr = xrpool[c].tile([128, 512], F32)
            g = t % 4
            col0 = (t // 4) * 256 + c * 128
            xrhs = xsb[32 * g : 32 * g + 32, col0 : col0 + 128]

            def xmm(m, out, start):
                lhsT = wi[32 * g : 32 * g + 32, m * 128 : (m + 1) * 128]
                nc.tensor.matmul(
                    out, lhsT, xrhs, start=start, stop=False,
                    tile_position=(32 * g, 0),
                )

            def bmm(j, out):
                # same row group as the x-MMs: serializes in-array (no
                # cross-group write race into a shared PSUM region)
                lhsT = bz[32 * g : 32 * g + 1, j * 128 : (j + 1) * 128]
                rhs = ones[32 * g : 32 * g + 1, 0:128]
                nc.tensor.matmul(
                    out, lhsT, rhs, start=False, stop=False,
                    tile_position=(32 * g, 0),
                )

            # zr bank: z0 z1 r0 r1 ; xr bank: xh0 xh1 | rh0 rh1
            xmm(0, zr[:, 0:128], True)       # z0 clears zr bank
            bmm(0, zr[:, 0:128])
            xmm(1, zr[:, 128:256], False)
            bmm(1, zr[:, 128:256])
            xmm(2, zr[:, 256:384], False)
            bmm(2, zr[:, 256:384])
            xmm(3, zr[:, 384:512], False)
            bmm(3, zr[:, 384:512])
            xmm(4, xr[:, 0:128], True)       # xh0 clears xr bank
            xmm(5, xr[:, 128:256], False)
            return zr, xr

        cur = [x_phase(0, 0), x_phase(0, 1)]

        for t in range(T):
            for c in (0, 1):
                zr, xr = cur[c]
                hc = h[c]
                # rec matmuls: r (m=2,3) first, then rh (m=4,5), then z (m=0,1)
                for m in (2, 3, 4, 5, 0, 1):
                    if m < 4:
                        out = zr[:, m * 128 : (m + 1) * 128]
                    else:
                        out = xr[:, 256 + (m - 4) * 128 : 256 + (m - 3) * 128]
                    for ct in (0, 1):
                        lhsT = wr[:, (ct * 6 + m) * 128 : (ct * 6 + m + 1) * 128]
                        rhs = hc[:, ct * 128 : (ct + 1) * 128]
                        stop = (ct == 1) and (m in (1, 5))
                        nc.tensor.matmul(out, lhsT, rhs, start=False, stop=stop)

                r_sb = gpool[c].tile([128, 256], F16, tag="r")
                z_sb = gpool[c].tile([128, 256], F16, tag="z")
                xh_sb = gpool[c].tile([128, 256], F16, tag="xh")
                nc.scalar.activation(r_sb[:], zr[:, 256:512], AF.Sigmoid)
                nc.scalar.activation(xh_sb[:], xr[:, 0:256], AF.Copy)
                nc.scalar.activation(z_sb[:], zr[:, 0:256], AF.Sigmoid)

                p = gpool[c].tile([128, 256], F16, tag="p")
                for ct in (0, 1):
                    nc.vector.scalar_tensor_tensor(
                        p[:, ct * 128 : (ct + 1) * 128],
                        xr[:, 256 + ct * 128 : 256 + (ct + 1) * 128],
                        brh[:, ct : ct + 1],
                        r_sb[:, ct * 128 : (ct + 1) * 128],
                        op0=mybir.AluOpType.add,
                        op1=mybir.AluOpType.mult,
                    )
                pre = gpool[c].tile([128, 256], F16, tag="pre")
                nc.vector.tensor_add(pre[:], xh_sb[:], p[:])
                hh = gpool[c].tile([128, 256], F16, tag="hh")
                for ct in (0, 1):
                    nc.vector.tensor_scalar(
                        hh[:, ct * 128 : (ct + 1) * 128],
                        pre[:, ct * 128 : (ct + 1) * 128],
                        bxh[:, ct : ct + 1],
                        0.0,
                        op0=mybir.AluOpType.add,
                        op1=mybir.AluOpType.max,
                    )
                d = gpool[c].tile([128, 256], F16, tag="d")
                nc.vector.tensor_sub(d[:], hc[:], hh[:])
                e = gpool[c].tile([128, 256], F16, tag="e")
                nc.vector.tensor_mul(e[:], z_sb[:], d[:])
                hn = hpool[c].tile([128, 256], F16)
                nc.vector.tensor_add(hn[:], hh[:], e[:])
                h[c] = hn

                if t + 1 < T:
                    cur[c] = x_phase(t + 1, c)

        # ---- BN + dense epilogue: y = s . h + c ----
        fin = zrpool[0].tile([128, 512], F32, tag="zr")
        first = True
        for c in (0, 1):
            for ct in (0, 1):
                nc.tensor.matmul(
                    fin[0:1, c * 128 : (c + 1) * 128],
                    sv[:, ct : ct + 1],
                    h[c][:, ct * 128 : (ct + 1) * 128],
                    start=first,
                    stop=(c == 1 and ct == 1),
                )
                first = False
        ysb = const.tile([1, 256], F32)
        nc.vector.tensor_scalar_add(ysb[:], fin[0:1, 0:256], cv[0:1, 0:1])
        nc.sync.dma_start(y_d.ap(), ysb[:])

    nc.compile()
    return nc


BN_EPS = 1e-3


def prep_core_inputs(x_core, kernel, rec_kernel, bias, gamma, beta,
                     moving_mean, moving_var, dense_w, dense_b):
    """Host-side prep of one core's input dict. x_core: [B=256, T, 32] f32."""
    import numpy as np
    B, T, F = x_core.shape
    H = 256
    xT = np.ascontiguousarray(
        x_core.astype(np.float16).transpose(1, 2, 0).reshape(T * F, B)
    )
    rec = rec_kernel.astype(np.float16)
    wr = np.concatenate(
        [rec[ct * 128 : (ct + 1) * 128, m * 128 : (m + 1) * 128]
         for ct in (0, 1) for m in range(6)],
        axis=1,
    )
    wi = np.tile(kernel.astype(np.float16), (4, 1))
    b_z = bias[0, 0:256] + bias[1, 0:256]
    b_r = bias[0, 256:512] + bias[1, 256:512]
    b_xh = bias[0, 512:768]
    b_rh = bias[1, 512:768]
    b_all = np.concatenate([b_z, b_r]).astype(np.float16)
    bz = np.zeros((128, 1024), np.float16)
    for g in (0, 32, 64, 96):
        bz[g, 0:512] = b_all
    brh = np.stack([b_rh[:128], b_rh[128:]], axis=1).astype(np.float32)
    bxh = np.stack([b_xh[:128], b_xh[128:]], axis=1).astype(np.float32)
    rs = 1.0 / np.sqrt(moving_var + BN_EPS)
    s = (gamma * rs * dense_w[:, 0]).astype(np.float16)
    sv = np.stack([s[:128], s[128:]], axis=1)
    cc = dense_b[0] + np.sum((beta - moving_mean * gamma * rs) * dense_w[:, 0])
    cv = np.array([[cc]], np.float32)
    return {
        "xT": np.ascontiguousarray(xT),
        "wr": np.ascontiguousarray(wr),
        "wi": np.ascontiguousarray(wi),
        "bz": bz,
        "sv": np.ascontiguousarray(sv),
        "cv": cv,
        "brh": np.ascontiguousarray(brh),
        "bxh": np.ascontiguousarray(bxh),
    }


_NC_CACHE = {}


def _get_nc():
    if "nc" not in _NC_CACHE:
        _NC_CACHE["nc"] = build_gru_nc(T=256)
    return _NC_CACHE["nc"]


def kernel(x, kernel, rec_kernel, bias, gamma, beta, moving_mean, moving_var,
           dense_w, dense_b):
    from concourse.bass_utils import run_bass_kernel_spmd

    x = np.asarray(x, dtype=np.float32)
    args = [np.asarray(a, dtype=np.float32) for a in
            (kernel, rec_kernel, bias, gamma, beta, moving_mean, moving_var,
             dense_w, dense_b)]
    nc = _get_nc()
    n_cores = 8
    nb = x.shape[0] // n_cores
    in_maps = [prep_core_inputs(x[i * nb : (i + 1) * nb], *args)
               for i in range(n_cores)]
    res = run_bass_kernel_spmd(nc, in_maps, core_ids=list(range(n_cores)))
    return np.concatenate(
        [res.results[i]["y"].reshape(nb, 1) for i in range(n_cores)], axis=0
    ).astype(np.float32)



# revision 17
# speedup vs baseline: 1.1837x; 1.0509x over previous
"""Self-contained Trainium2 Bass kernel: GRU(relu, reset_after) + BN + Dense.

kernel(**inputs) takes FULL unsharded fp32 inputs, shards batch over 8
NeuronCores, runs the Bass kernel via run_bass_kernel_spmd, returns the
FULL [2048, 1] fp32 output.
"""
import numpy as np

"""GRU Bass kernel builder for TRN2 (one NeuronCore program, SPMD over 8 cores).

Layout (per core):
  B=256 batch (2 chunks of Bc=128), T timesteps, F=32 in-features, H=256 hidden.
  Everything transposed: H on partitions, batch on free dim.

DRAM inputs (host-prepped, fp16 unless noted):
  xT   [T*32, 256]  x transposed+interleaved: row t*32+f, col b
  wr   [128, 1536]  rec_kernel lhsT tiles: col block (c*6+m)*128 = rec[c*128:., m*128:.]
  wi   [128, 768]   input kernel replicated at partition groups 0/32/64/96
  bz   [128, 1024]  bias rows at partitions {0,32,64,96}; col block j = b_all[j*128:(j+1)*128]
                    b_all = [b_z(256) | b_r(256) | b_xh(256) | b_rh(256)]
  sv   [128, 2]     BN+dense folded scale s: col c = s[c*128:(c+1)*128]
  cv   [1, 1] f32   scalar constant folded from BN/dense biases
Output:
  y    [1, 256] f32  per-core output slice (before host concat)
"""
from contextlib import ExitStack

import concourse.bass as bass
import concourse.tile as tile
from concourse import bacc, mybir

F16 = mybir.dt.float16
F32 = mybir.dt.float32
AF = mybir.ActivationFunctionType


def build_gru_nc(T=256, debug=False):
    nc = bacc.Bacc("TRN2", num_devices=8, debug=debug)
    xT_d = nc.dram_tensor("xT", [T * 32, 256], F16, kind="ExternalInput")
    wr_d = nc.dram_tensor("wr", [128, 1536], F16, kind="ExternalInput")
    wi_d = nc.dram_tensor("wi", [128, 768], F16, kind="ExternalInput")
    bzr_d = nc.dram_tensor("bzr", [128, 4], F32, kind="ExternalInput")
    sv_d = nc.dram_tensor("sv", [128, 2], F16, kind="ExternalInput")
    cv_d = nc.dram_tensor("cv", [1, 1], F32, kind="ExternalInput")
    brh_d = nc.dram_tensor("brh", [128, 2], F32, kind="ExternalInput")
    bxh_d = nc.dram_tensor("bxh", [128, 2], F32, kind="ExternalInput")
    y_d = nc.dram_tensor("y", [1, 256], F32, kind="ExternalOutput")

    with tile.TileContext(nc) as tc, ExitStack() as ctx:
        const = ctx.enter_context(tc.tile_pool(name="const", bufs=1))
        hpool = [
            ctx.enter_context(tc.tile_pool(name=f"h{c}", bufs=2)) for c in (0, 1)
        ]
        gpool = [
            ctx.enter_context(tc.tile_pool(name=f"g{c}", bufs=2)) for c in (0, 1)
        ]
        zrpool = [
            ctx.enter_context(
                tc.tile_pool(name=f"zr{c}", bufs=2, space=bass.MemorySpace.PSUM)
            )
            for c in (0, 1)
        ]
        xrpool = [
            ctx.enter_context(
                tc.tile_pool(name=f"xr{c}", bufs=2, space=bass.MemorySpace.PSUM)
            )
            for c in (0, 1)
        ]

        # ---- constants / weights ----
        xsb = const.tile([128, (T * 32 // 128) * 256], F16)  # x^T tiles, col blk j
        wr = const.tile([128, 1536], F16)
        wi = const.tile([128, 768], F16)
        bzr = const.tile([128, 4], F32)
        sv = const.tile([128, 2], F16)
        cv = const.tile([1, 1], F32)
        brh = const.tile([128, 2], F32)
        bxh = const.tile([128, 2], F32)

        nc.sync.dma_start(wr[:], wr_d.ap())
        nc.sync.dma_start(wi[:], wi_d.ap())
        nc.sync.dma_start(bzr[:], bzr_d.ap())
        nc.sync.dma_start(sv[:], sv_d.ap())
        nc.sync.dma_start(cv[:], cv_d.ap())
        nc.sync.dma_start(brh[:], brh_d.ap())
        nc.sync.dma_start(bxh[:], bxh_d.ap())

        # x load: split into up to 4 chunks of row-tiles for pipelining
        ntile = T * 32 // 128  # 64 for T=256
        nchunk = min(4, ntile)
        per = ntile // nchunk
        for jc in range(nchunk):
            src = xT_d.ap()[jc * per * 128 : (jc + 1) * per * 128, :]
            src = src.rearrange("(j p) b -> p j b", p=128)
            dst = xsb[:, jc * per * 256 : (jc + 1) * per * 256]
            dst = dst.rearrange("p (j b) -> p j b", b=256)
            nc.sync.dma_start(dst, src)

        # ---- initial hidden state ----
        h = []
        for c in (0, 1):
            h0 = hpool[c].tile([128, 256], F16)
            nc.vector.memset(h0[:], 0.0)
            h.append(h0)

        def x_phase(t, c):
            """Emit x-projection + bias matmuls for step t, chunk c."""
            zr = zrpool[c].tile([128, 512], F32)
            xr = xrpool[c].tile([128, 512], F32)
            g = t % 4
            col0 = (t // 4) * 256 + c * 128
            xrhs = xsb[32 * g : 32 * g + 32, col0 : col0 + 128]

            def xmm(m, out, start):
                lhsT = wi[32 * g : 32 * g + 32, m * 128 : (m + 1) * 128]
                nc.tensor.matmul(
                    out, lhsT, xrhs, start=start, stop=False,
                    tile_position=(32 * g, 0),
                )

            # zr bank: z0 z1 r0 r1 ; xr bank: xh0 xh1 | rh0 rh1
            xmm(0, zr[:, 0:128], True)       # z0 clears zr bank
            xmm(1, zr[:, 128:256], False)
            xmm(2, zr[:, 256:384], False)
            xmm(3, zr[:, 384:512], False)
            xmm(4, xr[:, 0:128], True)       # xh0 clears xr bank
            xmm(5, xr[:, 128:256], False)
            return zr, xr

        cur = [x_phase(0, 0), x_phase(0, 1)]

        for t in range(T):
            for c in (0, 1):
                zr, xr = cur[c]
                hc = h[c]
                # rec matmuls: r (m=2,3) first, then rh (m=4,5), then z (m=0,1)
                for m in (2, 3, 4, 5, 0, 1):
                    if m < 4:
                        out = zr[:, m * 128 : (m + 1) * 128]
                    else:
                        out = xr[:, 256 + (m - 4) * 128 : 256 + (m - 3) * 128]
                    for ct in (0, 1):
                        lhsT = wr[:, (ct * 6 + m) * 128 : (ct * 6 + m + 1) * 128]
                        rhs = hc[:, ct * 128 : (ct + 1) * 128]
                        stop = (ct == 1) and (m in (1, 5))
                        nc.tensor.matmul(out, lhsT, rhs, start=False, stop=stop)

                r_sb = gpool[c].tile([128, 256], F16, tag="r")
                z_sb = gpool[c].tile([128, 256], F16, tag="z")
                for ct in (0, 1):
                    nc.scalar.activation(
                        r_sb[:, ct * 128 : (ct + 1) * 128],
                        zr[:, 256 + ct * 128 : 256 + (ct + 1) * 128],
                        AF.Sigmoid, bias=bzr[:, 2 + ct : 3 + ct],
                    )
                for ct in (0, 1):
                    nc.scalar.activation(
                        z_sb[:, ct * 128 : (ct + 1) * 128],
                        zr[:, ct * 128 : (ct + 1) * 128],
                        AF.Sigmoid, bias=bzr[:, ct : ct + 1],
                    )

                w_sb = gpool[c].tile([128, 256], F16, tag="w")
                nc.gpsimd.tensor_scalar(
                    w_sb[:], z_sb[:], -1.0, 1.0,
                    op0=mybir.AluOpType.mult, op1=mybir.AluOpType.add,
                )
                u = gpool[c].tile([128, 256], F16, tag="u")
                nc.gpsimd.tensor_tensor(
                    u[:], z_sb[:], hc[:], mybir.AluOpType.mult
                )

                p = gpool[c].tile([128, 256], F16, tag="p")
                for ct in (0, 1):
                    nc.vector.scalar_tensor_tensor(
                        p[:, ct * 128 : (ct + 1) * 128],
                        xr[:, 256 + ct * 128 : 256 + (ct + 1) * 128],
                        brh[:, ct : ct + 1],
                        r_sb[:, ct * 128 : (ct + 1) * 128],
                        op0=mybir.AluOpType.add,
                        op1=mybir.AluOpType.mult,
                    )
                # s = (xh_psum + b_xh) + p, straight from PSUM
                s = gpool[c].tile([128, 256], F16, tag="s")
                for ct in (0, 1):
                    nc.vector.scalar_tensor_tensor(
                        s[:, ct * 128 : (ct + 1) * 128],
                        xr[:, ct * 128 : (ct + 1) * 128],
                        bxh[:, ct : ct + 1],
                        p[:, ct * 128 : (ct + 1) * 128],
                        op0=mybir.AluOpType.add,
                        op1=mybir.AluOpType.add,
                    )
                # q = s*(1-z); hn = relu(q) + z*h  (relu commutes: 1-z > 0)
                q = gpool[c].tile([128, 256], F16, tag="q")
                nc.vector.tensor_mul(q[:], s[:], w_sb[:])
                hn = hpool[c].tile([128, 256], F16)
                nc.vector.scalar_tensor_tensor(
                    hn[:], q[:], 0.0, u[:],
                    op0=mybir.AluOpType.max, op1=mybir.AluOpType.add,
                )
                h[c] = hn

                if t + 1 < T:
                    cur[c] = x_phase(t + 1, c)

        # ---- BN + dense epilogue: y = s . h + c ----
        fin = zrpool[0].tile([128, 512], F32, tag="zr")
        first = True
        for c in (0, 1):
            for ct in (0, 1):
                nc.tensor.matmul(
                    fin[0:1, c * 128 : (c + 1) * 128],
                    sv[:, ct : ct + 1],
                    h[c][:, ct * 128 : (ct + 1) * 128],
                    start=first,
                    stop=(c == 1 and ct == 1),
                )
                first = False
        ysb = const.tile([1, 256], F32)
        nc.vector.tensor_scalar_add(ysb[:], fin[0:1, 0:256], cv[0:1, 0:1])
        nc.sync.dma_start(y_d.ap(), ysb[:])

    nc.compile()
    return nc


BN_EPS = 1e-3


def prep_core_inputs(x_core, kernel, rec_kernel, bias, gamma, beta,
                     moving_mean, moving_var, dense_w, dense_b):
    """Host-side prep of one core's input dict. x_core: [B=256, T, 32] f32."""
    import numpy as np
    B, T, F = x_core.shape
    H = 256
    xT = np.ascontiguousarray(
        x_core.astype(np.float16).transpose(1, 2, 0).reshape(T * F, B)
    )
    rec = rec_kernel.astype(np.float16)
    wr = np.concatenate(
        [rec[ct * 128 : (ct + 1) * 128, m * 128 : (m + 1) * 128]
         for ct in (0, 1) for m in range(6)],
        axis=1,
    )
    wi = np.tile(kernel.astype(np.float16), (4, 1))
    b_z = bias[0, 0:256] + bias[1, 0:256]
    b_r = bias[0, 256:512] + bias[1, 256:512]
    b_xh = bias[0, 512:768]
    b_rh = bias[1, 512:768]
    bzr = np.stack(
        [b_z[:128], b_z[128:], b_r[:128], b_r[128:]], axis=1
    ).astype(np.float32)
    brh = np.stack([b_rh[:128], b_rh[128:]], axis=1).astype(np.float32)
    bxh = np.stack([b_xh[:128], b_xh[128:]], axis=1).astype(np.float32)
    rs = 1.0 / np.sqrt(moving_var + BN_EPS)
    s = (gamma * rs * dense_w[:, 0]).astype(np.float16)
    sv = np.stack([s[:128], s[128:]], axis=1)
    cc = dense_b[0] + np.sum((beta - moving_mean * gamma * rs) * dense_w[:, 0])
    cv = np.array([[cc]], np.float32)
    return {
        "xT": np.ascontiguousarray(xT),
        "wr": np.ascontiguousarray(wr),
        "wi": np.ascontiguousarray(wi),
        "bzr": np.ascontiguousarray(bzr),
        "sv": np.ascontiguousarray(sv),
        "cv": cv,
        "brh": np.ascontiguousarray(brh),
        "bxh": np.ascontiguousarray(bxh),
    }


_NC_CACHE = {}


def _get_nc():
    if "nc" not in _NC_CACHE:
        _NC_CACHE["nc"] = build_gru_nc(T=256)
    return _NC_CACHE["nc"]


def kernel(x, kernel, rec_kernel, bias, gamma, beta, moving_mean, moving_var,
           dense_w, dense_b):
    from concourse.bass_utils import run_bass_kernel_spmd

    x = np.asarray(x, dtype=np.float32)
    args = [np.asarray(a, dtype=np.float32) for a in
            (kernel, rec_kernel, bias, gamma, beta, moving_mean, moving_var,
             dense_w, dense_b)]
    nc = _get_nc()
    n_cores = 8
    nb = x.shape[0] // n_cores
    in_maps = [prep_core_inputs(x[i * nb : (i + 1) * nb], *args)
               for i in range(n_cores)]
    res = run_bass_kernel_spmd(nc, in_maps, core_ids=list(range(n_cores)))
    return np.concatenate(
        [res.results[i]["y"].reshape(nb, 1) for i in range(n_cores)], axis=0
    ).astype(np.float32)



# revision 18
# speedup vs baseline: 1.2007x; 1.0143x over previous
"""Self-contained Trainium2 Bass kernel: GRU(relu, reset_after) + BN + Dense.

kernel(**inputs) takes FULL unsharded fp32 inputs, shards batch over 8
NeuronCores, runs the Bass kernel via run_bass_kernel_spmd, returns the
FULL [2048, 1] fp32 output.
"""
import numpy as np

"""GRU Bass kernel builder for TRN2 (one NeuronCore program, SPMD over 8 cores).

Layout (per core):
  B=256 batch (2 chunks of Bc=128), T timesteps, F=32 in-features, H=256 hidden.
  Everything transposed: H on partitions, batch on free dim.

DRAM inputs (host-prepped, fp16 unless noted):
  xT   [T*32, 256]  x transposed+interleaved: row t*32+f, col b
  wr   [128, 1536]  rec_kernel lhsT tiles: col block (c*6+m)*128 = rec[c*128:., m*128:.]
  wi   [128, 768]   input kernel replicated at partition groups 0/32/64/96
  bz   [128, 1024]  bias rows at partitions {0,32,64,96}; col block j = b_all[j*128:(j+1)*128]
                    b_all = [b_z(256) | b_r(256) | b_xh(256) | b_rh(256)]
  sv   [128, 2]     BN+dense folded scale s: col c = s[c*128:(c+1)*128]
  cv   [1, 1] f32   scalar constant folded from BN/dense biases
Output:
  y    [1, 256] f32  per-core output slice (before host concat)
"""
from contextlib import ExitStack

import concourse.bass as bass
import concourse.tile as tile
from concourse import bacc, mybir

F16 = mybir.dt.float16
F32 = mybir.dt.float32
AF = mybir.ActivationFunctionType


def build_gru_nc(T=256, debug=False):
    nc = bacc.Bacc("TRN2", num_devices=8, debug=debug)
    xT_d = nc.dram_tensor("xT", [T * 32, 256], F16, kind="ExternalInput")
    wr_d = nc.dram_tensor("wr", [128, 1536], F16, kind="ExternalInput")
    wi_d = nc.dram_tensor("wi", [128, 768], F16, kind="ExternalInput")
    bz_d = nc.dram_tensor("bz", [128, 1024], F16, kind="ExternalInput")
    sv_d = nc.dram_tensor("sv", [128, 2], F16, kind="ExternalInput")
    cv_d = nc.dram_tensor("cv", [1, 1], F32, kind="ExternalInput")
    brh_d = nc.dram_tensor("brh", [128, 2], F32, kind="ExternalInput")
    bxh_d = nc.dram_tensor("bxh", [128, 2], F32, kind="ExternalInput")
    y_d = nc.dram_tensor("y", [1, 256], F32, kind="ExternalOutput")

    with tile.TileContext(nc) as tc, ExitStack() as ctx:
        const = ctx.enter_context(tc.tile_pool(name="const", bufs=1))
        hpool = [
            ctx.enter_context(tc.tile_pool(name=f"h{c}", bufs=2)) for c in (0, 1)
        ]
        gpool = [
            ctx.enter_context(tc.tile_pool(name=f"g{c}", bufs=2)) for c in (0, 1)
        ]
        zrpool = [
            ctx.enter_context(
                tc.tile_pool(name=f"zr{c}", bufs=2, space=bass.MemorySpace.PSUM)
            )
            for c in (0, 1)
        ]
        xrpool = [
            ctx.enter_context(
                tc.tile_pool(name=f"xr{c}", bufs=2, space=bass.MemorySpace.PSUM)
            )
            for c in (0, 1)
        ]

        # ---- constants / weights ----
        xsb = const.tile([128, (T * 32 // 128) * 256], F16)  # x^T tiles, col blk j
        wr = const.tile([128, 1536], F16)
        wi = const.tile([128, 768], F16)
        bz = const.tile([128, 1024], F16)
        sv = const.tile([128, 2], F16)
        cv = const.tile([1, 1], F32)
        brh = const.tile([128, 2], F32)
        bxh = const.tile([128, 2], F32)
        ones = const.tile([128, 128], F16)

        nc.sync.dma_start(wr[:], wr_d.ap())
        nc.sync.dma_start(wi[:], wi_d.ap())
        nc.sync.dma_start(bz[:], bz_d.ap())
        nc.sync.dma_start(sv[:], sv_d.ap())
        nc.sync.dma_start(cv[:], cv_d.ap())
        nc.sync.dma_start(brh[:], brh_d.ap())
        nc.sync.dma_start(bxh[:], bxh_d.ap())
        nc.vector.memset(ones[:], 1.0)

        # x load: split into up to 4 chunks of row-tiles for pipelining
        ntile = T * 32 // 128  # 64 for T=256
        nchunk = min(4, ntile)
        per = ntile // nchunk
        for jc in range(nchunk):
            src = xT_d.ap()[jc * per * 128 : (jc + 1) * per * 128, :]
            src = src.rearrange("(j p) b -> p j b", p=128)
            dst = xsb[:, jc * per * 256 : (jc + 1) * per * 256]
            dst = dst.rearrange("p (j b) -> p j b", b=256)
            nc.sync.dma_start(dst, src)

        # ---- initial hidden state ----
        h = []
        for c in (0, 1):
            h0 = hpool[c].tile([128, 256], F16)
            nc.vector.memset(h0[:], 0.0)
            h.append(h0)

        def x_phase(t, c):
            """Emit x-projection + bias matmuls for step t, chunk c."""
            zr = zrpool[c].tile([128, 512], F32)
            xr = xrpool[c].tile([128, 512], F32)
            g = t % 4
            col0 = (t // 4) * 256 + c * 128
            xrhs = xsb[32 * g : 32 * g + 32, col0 : col0 + 128]

            def xmm(m, out, start):
                lhsT = wi[32 * g : 32 * g + 32, m * 128 : (m + 1) * 128]
                nc.tensor.matmul(
                    out, lhsT, xrhs, start=start, stop=False,
                    tile_position=(32 * g, 0),
                )

            def bmm(j, out):
                # same row group as the x-MMs: serializes in-array (no
                # cross-group write race into a shared PSUM region)
                lhsT = bz[32 * g : 32 * g + 1, j * 128 : (j + 1) * 128]
                rhs = ones[32 * g : 32 * g + 1, 0:128]
                nc.tensor.matmul(
                    out, lhsT, rhs, start=False, stop=False,
                    tile_position=(32 * g, 0),
                )

            # zr bank: z0 z1 r0 r1 ; xr bank: xh0 xh1 | rh0 rh1
            xmm(0, zr[:, 0:128], True)       # z0 clears zr bank
            bmm(0, zr[:, 0:128])
            xmm(1, zr[:, 128:256], False)
            bmm(1, zr[:, 128:256])
            xmm(2, zr[:, 256:384], False)
            bmm(2, zr[:, 256:384])
            xmm(3, zr[:, 384:512], False)
            bmm(3, zr[:, 384:512])
            xmm(4, xr[:, 0:128], True)       # xh0 clears xr bank
            xmm(5, xr[:, 128:256], False)
            return zr, xr

        cur = [x_phase(0, 0), x_phase(0, 1)]

        for t in range(T):
            for c in (0, 1):
                zr, xr = cur[c]
                hc = h[c]
                # rec matmuls: r (m=2,3) first, then rh (m=4,5), then z (m=0,1)
                for m in (2, 3, 4, 5, 0, 1):
                    if m < 4:
                        out = zr[:, m * 128 : (m + 1) * 128]
                    else:
                        out = xr[:, 256 + (m - 4) * 128 : 256 + (m - 3) * 128]
                    for ct in (0, 1):
                        lhsT = wr[:, (ct * 6 + m) * 128 : (ct * 6 + m + 1) * 128]
                        rhs = hc[:, ct * 128 : (ct + 1) * 128]
                        stop = (ct == 1) and (m in (1, 5))
                        nc.tensor.matmul(out, lhsT, rhs, start=False, stop=stop)

                r_sb = gpool[c].tile([128, 256], F16, tag="r")
                z_sb = gpool[c].tile([128, 256], F16, tag="z")
                xh_sb = gpool[c].tile([128, 256], F16, tag="xh")
                nc.scalar.activation(r_sb[:], zr[:, 256:512], AF.Sigmoid)
                nc.scalar.activation(xh_sb[:], xr[:, 0:256], AF.Copy)
                nc.scalar.activation(z_sb[:], zr[:, 0:256], AF.Sigmoid)
                w_sb = gpool[c].tile([128, 256], F16, tag="w")
                nc.gpsimd.tensor_scalar(
                    w_sb[:], z_sb[:], -1.0, 1.0,
                    op0=mybir.AluOpType.mult, op1=mybir.AluOpType.add,
                )
                u = gpool[c].tile([128, 256], F16, tag="u")
                nc.gpsimd.tensor_tensor(
                    u[:], z_sb[:], hc[:], mybir.AluOpType.mult
                )

                p = gpool[c].tile([128, 256], F16, tag="p")
                for ct in (0, 1):
                    nc.vector.scalar_tensor_tensor(
                        p[:, ct * 128 : (ct + 1) * 128],
                        xr[:, 256 + ct * 128 : 256 + (ct + 1) * 128],
                        brh[:, ct : ct + 1],
                        r_sb[:, ct * 128 : (ct + 1) * 128],
                        op0=mybir.AluOpType.add,
                        op1=mybir.AluOpType.mult,
                    )
                pre = gpool[c].tile([128, 256], F16, tag="pre")
                nc.vector.tensor_add(pre[:], xh_sb[:], p[:])
                hh = gpool[c].tile([128, 256], F16, tag="hh")
                for ct in (0, 1):
                    nc.vector.tensor_scalar(
                        hh[:, ct * 128 : (ct + 1) * 128],
                        pre[:, ct * 128 : (ct + 1) * 128],
                        bxh[:, ct : ct + 1],
                        0.0,
                        op0=mybir.AluOpType.add,
                        op1=mybir.AluOpType.max,
                    )
                k = gpool[c].tile([128, 256], F16, tag="k")
                nc.vector.tensor_mul(k[:], w_sb[:], hh[:])
                hn = hpool[c].tile([128, 256], F16)
                nc.vector.tensor_add(hn[:], k[:], u[:])
                h[c] = hn

                if t + 1 < T:
                    cur[c] = x_phase(t + 1, c)
                # dummy matmuls into the consumed z|r bank: keep the PE (and
                # its HAM clock gate) busy through the recurrence stall; the
                # garbage is erased by the next generation's start=True clear
                for dk in range(4):
                    nc.tensor.matmul(
                        zr[:, 0:512], wr[:, 0:128],
                        xsb[:, dk * 512 : dk * 512 + 512],
                        start=True, stop=True, skip_group_check=True,
                    )

        # ---- BN + dense epilogue: y = s . h + c ----
        fin = zrpool[0].tile([128, 512], F32, tag="zr")
        first = True
        for c in (0, 1):
            for ct in (0, 1):
                nc.tensor.matmul(
                    fin[0:1, c * 128 : (c + 1) * 128],
                    sv[:, ct : ct + 1],
                    h[c][:, ct * 128 : (ct + 1) * 128],
                    start=first,
                    stop=(c == 1 and ct == 1),
                )
                first = False
        ysb = const.tile([1, 256], F32)
        nc.vector.tensor_scalar_add(ysb[:], fin[0:1, 0:256], cv[0:1, 0:1])
        nc.sync.dma_start(y_d.ap(), ysb[:])

    nc.compile()
    return nc


BN_EPS = 1e-3


def prep_core_inputs(x_core, kernel, rec_kernel, bias, gamma, beta,
                     moving_mean, moving_var, dense_w, dense_b):
    """Host-side prep of one core's input dict. x_core: [B=256, T, 32] f32."""
    import numpy as np
    B, T, F = x_core.shape
    H = 256
    xT = np.ascontiguousarray(
        x_core.astype(np.float16).transpose(1, 2, 0).reshape(T * F, B)
    )
    rec = rec_kernel.astype(np.float16)
    wr = np.concatenate(
        [rec[ct * 128 : (ct + 1) * 128, m * 128 : (m + 1) * 128]
         for ct in (0, 1) for m in range(6)],
        axis=1,
    )
    wi = np.tile(kernel.astype(np.float16), (4, 1))
    b_z = bias[0, 0:256] + bias[1, 0:256]
    b_r = bias[0, 256:512] + bias[1, 256:512]
    b_xh = bias[0, 512:768]
    b_rh = bias[1, 512:768]
    b_all = np.concatenate([b_z, b_r]).astype(np.float16)
    bz = np.zeros((128, 1024), np.float16)
    for g in (0, 32, 64, 96):
        bz[g, 0:512] = b_all
    brh = np.stack([b_rh[:128], b_rh[128:]], axis=1).astype(np.float32)
    bxh = np.stack([b_xh[:128], b_xh[128:]], axis=1).astype(np.float32)
    rs = 1.0 / np.sqrt(moving_var + BN_EPS)
    s = (gamma * rs * dense_w[:, 0]).astype(np.float16)
    sv = np.stack([s[:128], s[128:]], axis=1)
    cc = dense_b[0] + np.sum((beta - moving_mean * gamma * rs) * dense_w[:, 0])
    cv = np.array([[cc]], np.float32)
    return {
        "xT": np.ascontiguousarray(xT),
        "wr": np.ascontiguousarray(wr),
        "wi": np.ascontiguousarray(wi),
        "bz": bz,
        "sv": np.ascontiguousarray(sv),
        "cv": cv,
        "brh": np.ascontiguousarray(brh),
        "bxh": np.ascontiguousarray(bxh),
    }


_NC_CACHE = {}


def _get_nc():
    if "nc" not in _NC_CACHE:
        _NC_CACHE["nc"] = build_gru_nc(T=256)
    return _NC_CACHE["nc"]


def kernel(x, kernel, rec_kernel, bias, gamma, beta, moving_mean, moving_var,
           dense_w, dense_b):
    from concourse.bass_utils import run_bass_kernel_spmd

    x = np.asarray(x, dtype=np.float32)
    args = [np.asarray(a, dtype=np.float32) for a in
            (kernel, rec_kernel, bias, gamma, beta, moving_mean, moving_var,
             dense_w, dense_b)]
    nc = _get_nc()
    n_cores = 8
    nb = x.shape[0] // n_cores
    in_maps = [prep_core_inputs(x[i * nb : (i + 1) * nb], *args)
               for i in range(n_cores)]
    res = run_bass_kernel_spmd(nc, in_maps, core_ids=list(range(n_cores)))
    return np.concatenate(
        [res.results[i]["y"].reshape(nb, 1) for i in range(n_cores)], axis=0
    ).astype(np.float32)

